# revision 6
# baseline (speedup 1.0000x reference)
"""Causal dot-product attention (B=4, S=4096, D=64) on 8 TRN2 NeuronCores.

Returns BOTH outputs of the reference: (attn_vec [B,S,D], attn_weights [B,S,S]).

Sharding: data-parallel over batch (4) x causal-balanced q-row interleave (2).
Core c handles batch b = c//2 and 16 of the 32 128-row q-blocks, chosen as
pairs (i, 31-i) so every core owns the same total causal area (load balance).

SPMD trick: the per-core *program* must be identical, but the causal widths of
a core's blocks differ between the two parities. The program computes padded
uniform widths W_k = 128*(2k+2) for the k-th (sorted) local block and applies a
per-core *data* mask (host-provided) to the boundary columns, which zeroes the
above-diagonal part exactly. Columns beyond W_k are never written: the PJRT
execution path donates zero-initialized output buffers, so the skipped region
is exactly 0.0 (= reference: exp((x-1e31)/8 - m) underflows to 0).

PE packing: the contraction dim is D=64, so two score matmuls are packed into
the 128x128 array as row-groups (tile_position (0,0) / (64,0)); Q^T and K^T are
host-duplicated onto partitions 64..127 to feed the second row-group. Each
packed pair fills a 2-bank PSUM tile -> one FD<=1024 exp on ScalarE.

Per-core pipeline:
  S^T pass: scores^T tiles [c128, q-chunk] = K^T-slice.T @ Q^T (PE, f32)
            -> exp(x/8) -> wT bf16 in SBUF (ACT), boundary sub-block masked
  PV pass:  vecT[64, 512] += V-slice.T @ wT-slice (PE, bf16), per 512-q window
            -> PE-transpose back to [q,64], scale by 1/rowsum (DVE)
  S pass:   scores tiles [q128, c-chunk] = Q^T-slice.T @ K^T (PE, f32)
            -> exp(x/8) f32 with accum_out row-sums (ACT) -> mask tail (DVE)
            -> scale by 1/rowsum (DVE) -> DMA out active columns only
"""

import os
import numpy as np
from contextlib import ExitStack

import ml_dtypes

from concourse import bacc, tile, mybir, masks
from concourse.bass_utils import run_bass_kernel_spmd

B, S, D = 4, 4096, 64
NCORES = 8
NLOCAL = 16          # 128-row q-blocks per core
SCALE = 0.125        # 1/sqrt(64)

BF16 = mybir.dt.bfloat16
F32 = mybir.dt.float32


def _block_list(parity: int) -> list[int]:
    """Global 128-row block ids owned by a core of this parity, sorted."""
    ids = []
    for i in range(16):
        if i % 2 == parity:
            ids.append(i)
            ids.append(31 - i)
    return sorted(ids)


def _padded_width(k: int) -> int:
    # uniform (parity-independent) padded causal width of local block k
    return 128 * (2 * k + 2)


def _pair_chunks(total: int) -> list[tuple[int, int, int]]:
    """Split [0, total) into (offset, w1, w2) packed-matmul pairs (w2 may be 0).
    Each wi <= 512; one pair -> one PSUM tile -> one exp of FD = w1 + w2."""
    out = []
    o = 0
    while o < total:
        w1 = min(512, total - o)
        w2 = min(512, total - o - w1)
        out.append((o, w1, w2))
        o += w1 + w2
    return out


def build_model(repeat: int = 1):
    nc = bacc.Bacc("TRN2", target_bir_lowering=False, debug=False,
                   num_devices=NCORES)

    # qt/kt carry two copies of the transposed tensor: partitions 0..63 and
    # 64..127 (feeds the second PE row-group of packed matmuls).
    qt_d = nc.dram_tensor("qt", [128, 2048], F32, kind="ExternalInput").ap()
    kt_d = nc.dram_tensor("kt", [128, S], F32, kind="ExternalInput").ap()
    v_d = nc.dram_tensor("v", [S, D], BF16, kind="ExternalInput").ap()
    smask_d = nc.dram_tensor("smask", [NLOCAL, 128, 256], F32,
                             kind="ExternalInput").ap()
    tmask_d = nc.dram_tensor("tmask", [32, 128, 128], BF16,
                             kind="ExternalInput").ap()
    w_out = nc.dram_tensor("w", [2048, S], F32, kind="ExternalOutput").ap()
    vec_out = nc.dram_tensor("vec", [2048, D], F32, kind="ExternalOutput").ap()

    trace_sim = os.environ.get("TILE_TRACE_SIM") == "1"
    with tile.TileContext(nc, trace_sim=trace_sim) as tc:
        with ExitStack() as ctx:
            const = ctx.enter_context(tc.tile_pool(name="const", bufs=1))

            # ---- load inputs ----
            qt = const.tile([128, 2048], F32)
            nc.sync.dma_start(qt[:], qt_d[:])
            kt = const.tile([128, S], F32)
            nc.sync.dma_start(kt[:], kt_d[:])
            vs = const.tile([128, 32 * D], BF16)
            nc.sync.dma_start(
                vs[:].rearrange("p (t d) -> p t d", t=32),
                v_d.rearrange("(t p) d -> p t d", p=128),
            )
            sm = const.tile([128, NLOCAL * 256], F32)
            nc.sync.dma_start(
                sm[:].rearrange("p (k j) -> p k j", k=NLOCAL),
                smask_d.rearrange("k p j -> p k j"),
            )
            tm = const.tile([128, 32 * 128], BF16)
            nc.sync.dma_start(
                tm[:].rearrange("p (c j) -> p c j", c=32),
                tmask_d.rearrange("c p j -> p c j"),
            )
            ident = const.tile([128, 128], F32)
            masks.make_identity(nc, ident[:])

            for rep in range(repeat):
                _emit_body(nc, tc, ctx, rep, qt, kt, vs, sm, tm, ident,
                           w_out, vec_out)

    nc.compile()
    return nc


def _emit_body(nc, tc, octx, rep, qt, kt, vs, sm, tm, ident, w_out, vec_out):
    with ExitStack() as ctx:
        wtp = ctx.enter_context(tc.tile_pool(name=f"wt{rep}", bufs=1))
        stg = ctx.enter_context(tc.tile_pool(name=f"stg{rep}", bufs=2))
        vtp = ctx.enter_context(tc.tile_pool(name=f"vt{rep}", bufs=2))
        vcp = ctx.enter_context(tc.tile_pool(name=f"vc{rep}", bufs=2))
        smallp = ctx.enter_context(tc.tile_pool(name=f"small{rep}", bufs=4))
        rp = ctx.enter_context(tc.tile_pool(name=f"rall{rep}", bufs=1))
        ps_sc = ctx.enter_context(
            tc.tile_pool(name=f"ps_sc{rep}", bufs=3, space="PSUM"))
        ps_vt = ctx.enter_context(
            tc.tile_pool(name=f"ps_vt{rep}", bufs=1, space="PSUM"))
        ps_tr = ctx.enter_context(
            tc.tile_pool(name=f"ps_tr{rep}", bufs=1, space="PSUM"))

        r_all = rp.tile([128, NLOCAL], F32)   # 1/rowsum per local block
        wt = {}  # ci -> bf16 tile [128, suffix]

        def emit_st_row(ci: int):
            """S^T pass for c-row ci: wT[ci] = exp(scores^T / 8) (bf16)."""
            kmin = ci // 2
            sfx = 2048 - 128 * kmin
            base = 128 * kmin
            t = wtp.tile([128, sfx], BF16, tag=f"wt{ci}")
            wt[ci] = t
            for (o, w1, w2) in _pair_chunks(sfx):
                fd = w1 + w2
                ps = ps_sc.tile([128, 1024], F32, tag="score")
                nc.tensor.matmul(
                    ps[:, 0:w1],
                    kt[0:64, 128 * ci:128 * ci + 128],
                    qt[0:64, base + o:base + o + w1],
                    start=True, stop=True,
                )
                if w2:
                    nc.tensor.matmul(
                        ps[:, 512:512 + w2],
                        kt[64:128, 128 * ci:128 * ci + 128],
                        qt[64:128, base + o + w1:base + o + w1 + w2],
                        start=True, stop=True,
                        tile_position=(64, 0),
                    )
                nc.scalar.activation(
                    t[:, o:o + fd], ps[:, 0:fd],
                    mybir.ActivationFunctionType.Exp, scale=SCALE,
                )
            # boundary sub-block: zero/tri/dense fixup (data-dependent mask)
            nc.vector.tensor_mul(
                t[:, 0:128], t[:, 0:128], tm[:, 128 * ci:128 * ci + 128])

        def emit_s_block(k: int):
            """S pass for local q-block k: normalized weights -> HBM."""
            W = _padded_width(k)
            stage = stg.tile([128, S], F32, tag="stage")
            partials = smallp.tile([128, 8], F32, tag="partials")
            prs = _pair_chunks(W)
            for cc, (o, w1, w2) in enumerate(prs):
                fd = w1 + w2
                ps = ps_sc.tile([128, 1024], F32, tag="score")
                nc.tensor.matmul(
                    ps[:, 0:w1],
                    qt[0:64, 128 * k:128 * k + 128],
                    kt[0:64, o:o + w1],
                    start=True, stop=True,
                )
                if w2:
                    nc.tensor.matmul(
                        ps[:, 512:512 + w2],
                        qt[64:128, 128 * k:128 * k + 128],
                        kt[64:128, o + w1:o + w1 + w2],
                        start=True, stop=True,
                        tile_position=(64, 0),
                    )
                nc.scalar.activation(
                    stage[:, o:o + fd], ps[:, 0:fd],
                    mybir.ActivationFunctionType.Exp, scale=SCALE,
                    accum_out=partials[:, cc:cc + 1],
                )
            # mask last 256 columns (triangle + padding) to exact 0
            nc.vector.tensor_mul(
                stage[:, W - 256:W], stage[:, W - 256:W],
                sm[:, 256 * k:256 * k + 256])
            # last pair's partial included masked garbage; recompute it
            o_l = prs[-1][0]
            fd_l = prs[-1][1] + prs[-1][2]
            ncc = len(prs)
            nc.vector.reduce_sum(
                out=partials[:, ncc - 1:ncc],
                in_=stage[:, o_l:o_l + fd_l],
                axis=mybir.AxisListType.X,
            )
            nc.vector.reduce_sum(
                out=partials[:, 7:8], in_=partials[:, 0:ncc],
                axis=mybir.AxisListType.X,
            )
            nc.vector.reciprocal(r_all[:, k:k + 1], partials[:, 7:8])
            nc.vector.tensor_scalar_mul(
                stage[:, 0:W], stage[:, 0:W], r_all[:, k:k + 1])
            nc.sync.dma_start(
                w_out[128 * k:128 * k + 128, 0:W], stage[:, 0:W])

        def emit_pv_window(w: int):
            """PV for local q window [512w, 512w+512): vec rows + DMA."""
            ci_hi = min(31, 8 * w + 7)
            pv = ps_vt.tile([64, 512], F32)
            n_ci = ci_hi + 1
            for ci in range(n_ci):
                o = 512 * w - 128 * (ci // 2)
                lhs = vs[:, 64 * ci:64 * ci + 64]
                if o >= 0:
                    nc.tensor.matmul(
                        pv[:, 0:512], lhs, wt[ci][:, o:o + 512],
                        start=(ci == 0), stop=(ci == n_ci - 1),
                    )
                else:
                    nc.tensor.matmul(
                        pv[:, -o:512], lhs, wt[ci][:, 0:512 + o],
                        start=False, stop=(ci == n_ci - 1),
                    )
            vt_sb = vtp.tile([64, 512], F32, tag="vtsb")
            nc.vector.tensor_copy(vt_sb[:], pv[:])
            for j in range(4):
                k = 4 * w + j
                tr = ps_tr.tile([128, 64], F32)
                nc.tensor.transpose(
                    tr[:], vt_sb[:, 128 * j:128 * j + 128],
                    ident[0:64, 0:64])
                vec_sb = vcp.tile([128, 64], F32, tag="vecsb")
                nc.vector.tensor_scalar_mul(
                    vec_sb[:], tr[:], r_all[:, k:k + 1])
                nc.sync.dma_start(
                    vec_out[128 * k:128 * k + 128, :], vec_sb[:])

        # ---- emission order: interleave for pipelining ----
        for u in range(NLOCAL):
            emit_st_row(2 * u)
            emit_st_row(2 * u + 1)
            emit_s_block(u)
            if u % 4 == 3:
                emit_pv_window(u // 4)


_NC = None


def _get_model():
    global _NC
    if _NC is None:
        _NC = build_model()
    return _NC


def _host_masks(parity: int):
    gl = _block_list(parity)
    smask = np.zeros((NLOCAL, 128, 256), dtype=np.float32)
    for k, g in enumerate(gl):
        W = _padded_width(k)
        cols = W - 256 + np.arange(256)[None, :]
        rows = 128 * g + np.arange(128)[:, None]
        smask[k] = (cols <= rows).astype(np.float32)
    tmask = np.zeros((32, 128, 128), dtype=np.float32)
    for ci in range(32):
        kmin = ci // 2
        g = gl[kmin]
        rows_c = 128 * ci + np.arange(128)[:, None]
        cols_q = 128 * g + np.arange(128)[None, :]
        tmask[ci] = (rows_c <= cols_q)
    return smask, tmask.astype(ml_dtypes.bfloat16)


def make_in_maps(query, key, value):
    query = np.asarray(query, dtype=np.float32)
    key = np.asarray(key, dtype=np.float32)
    value = np.asarray(value, dtype=np.float32)
    mask_cache = {p: _host_masks(p) for p in (0, 1)}
    in_maps = []
    for c in range(NCORES):
        b, p = c // 2, c % 2
        gl = _block_list(p)
        rows = np.concatenate(
            [np.arange(128 * g, 128 * g + 128) for g in gl])
        qt1 = query[b][rows].T                                # [64, 2048]
        kt1 = key[b].T                                        # [64, 4096]
        qt = np.ascontiguousarray(np.concatenate([qt1, qt1], axis=0))
        kt = np.ascontiguousarray(np.concatenate([kt1, kt1], axis=0))
        v = value[b].astype(ml_dtypes.bfloat16)               # [4096, 64]
        smask, tmask = mask_cache[p]
        in_maps.append(
            {"qt": qt, "kt": kt, "v": v, "smask": smask, "tmask": tmask})
    return in_maps


def kernel(query, key, value):
    nc = _get_model()
    in_maps = make_in_maps(query, key, value)
    res = run_bass_kernel_spmd(nc, in_maps, core_ids=list(range(NCORES)))

    attn_vec = np.empty((B, S, D), dtype=np.float32)
    attn_w = np.empty((B, S, S), dtype=np.float32)
    for c in range(NCORES):
        b, p = c // 2, c % 2
        gl = _block_list(p)
        wsh = res.results[c]["w"]
        vsh = res.results[c]["vec"]
        for k, g in enumerate(gl):
            attn_w[b, 128 * g:128 * g + 128, :] = wsh[128 * k:128 * k + 128]
            attn_vec[b, 128 * g:128 * g + 128, :] = vsh[128 * k:128 * k + 128]
    return attn_vec, attn_w


# revision 16
# speedup vs baseline: 1.0917x; 1.0917x over previous
"""Causal dot-product attention (B=4, S=4096, D=64) on 8 TRN2 NeuronCores.

Returns BOTH outputs of the reference: (attn_vec [B,S,D], attn_weights [B,S,S]).

Sharding: data-parallel over batch (4) x causal-balanced q-row interleave (2).
Core c handles batch b = c//2 and 16 of the 32 128-row q-blocks, chosen as
pairs (i, 31-i) so every core owns the same total causal area (load balance).

SPMD trick: the per-core *program* must be identical, but the causal widths of
a core's blocks differ between the two parities. The program computes padded
uniform widths W_k = 128*(2k+2) for the k-th (sorted) local block and applies a
per-core *data* mask (host-provided) to the boundary columns, which zeroes the
above-diagonal part exactly. Columns beyond W_k are never written: the PJRT
execution path donates zero-initialized output buffers, so the skipped region
is exactly 0.0 (= reference: exp((x-1e31)/8 - m) underflows to 0).

PE packing: the contraction dim is D=64, so two score matmuls are packed into
the 128x128 array as row-groups (tile_position (0,0) / (64,0)); Q^T and K^T are
host-duplicated onto partitions 64..127 to feed the second row-group. Each
packed pair fills a 2-bank PSUM tile -> one FD<=1024 exp on ScalarE.

Per-core pipeline:
  S^T pass: scores^T tiles [c128, q-chunk] = K^T-slice.T @ Q^T (PE, f32)
            -> exp(x/8) -> wT bf16 in SBUF (ACT), boundary sub-block masked
  PV pass:  vecT[64, 512] += V-slice.T @ wT-slice (PE, bf16), per 512-q window
            -> PE-transpose back to [q,64], scale by 1/rowsum (DVE)
  S pass:   scores tiles [q128, c-chunk] = Q^T-slice.T @ K^T (PE, f32)
            -> exp(x/8) f32 with accum_out row-sums (ACT) -> mask tail (DVE)
            -> scale by 1/rowsum (DVE) -> DMA out active columns only
"""

import os
import numpy as np
from contextlib import ExitStack

import ml_dtypes

from concourse import bacc, tile, mybir, masks
from concourse.bass_utils import run_bass_kernel_spmd

B, S, D = 4, 4096, 64
NCORES = 8
NLOCAL = 16          # 128-row q-blocks per core
SCALE = 0.125        # 1/sqrt(64)

BF16 = mybir.dt.bfloat16
F32 = mybir.dt.float32


def _block_list(parity: int) -> list[int]:
    """Global 128-row block ids owned by a core of this parity, sorted."""
    ids = []
    for i in range(16):
        if i % 2 == parity:
            ids.append(i)
            ids.append(31 - i)
    return sorted(ids)


def _padded_width(k: int) -> int:
    # uniform (parity-independent) padded causal width of local block k
    return 128 * (2 * k + 2)


def _pair_chunks(total: int) -> list[tuple[int, int, int]]:
    """Split [0, total) into (offset, w1, w2) packed-matmul pairs (w2 may be 0).
    Each wi <= 512; one pair -> one PSUM tile -> one exp of FD = w1 + w2."""
    out = []
    o = 0
    while o < total:
        w1 = min(512, total - o)
        w2 = min(512, total - o - w1)
        out.append((o, w1, w2))
        o += w1 + w2
    return out


def build_model(repeat: int = 1, loop: int = 1):
    nc = bacc.Bacc("TRN2", target_bir_lowering=False, debug=False,
                   num_devices=NCORES)

    # qt/kt carry two copies of the transposed tensor: partitions 0..63 and
    # 64..127 (feeds the second PE row-group of packed matmuls).
    qt_d = nc.dram_tensor("qt", [128, 2048], F32, kind="ExternalInput").ap()
    kt_d = nc.dram_tensor("kt", [128, S], F32, kind="ExternalInput").ap()
    # V with an appended ones column: the PV matmul then yields row-sums
    # (for vec normalization) in output partition 64 for free.
    v_d = nc.dram_tensor("v", [S, D + 1], BF16, kind="ExternalInput").ap()
    smask_d = nc.dram_tensor("smask", [NLOCAL, 128, 256], BF16,
                             kind="ExternalInput").ap()
    tmask_d = nc.dram_tensor("tmask", [32, 128, 128], BF16,
                             kind="ExternalInput").ap()
    w_out = nc.dram_tensor("w", [2048, S], F32, kind="ExternalOutput").ap()
    vec_out = nc.dram_tensor("vec", [2048, D], F32, kind="ExternalOutput").ap()

    trace_sim = os.environ.get("TILE_TRACE_SIM") == "1"
    with tile.TileContext(nc, trace_sim=trace_sim) as tc:
        with ExitStack() as ctx:
            const = ctx.enter_context(tc.tile_pool(name="const", bufs=1))

            # ---- load inputs ----
            qt = const.tile([128, 2048], F32)
            nc.sync.dma_start(qt[:], qt_d[:])
            kt = const.tile([128, S], F32)
            nc.sync.dma_start(kt[:], kt_d[:])
            vs = const.tile([128, 32 * (D + 1)], BF16)
            nc.sync.dma_start(
                vs[:].rearrange("p (t d) -> p t d", t=32),
                v_d.rearrange("(t p) d -> p t d", p=128),
            )
            sm = const.tile([128, NLOCAL * 256], BF16)
            nc.sync.dma_start(
                sm[:].rearrange("p (k j) -> p k j", k=NLOCAL),
                smask_d.rearrange("k p j -> p k j"),
            )
            tm = const.tile([128, 32 * 128], BF16)
            nc.sync.dma_start(
                tm[:].rearrange("p (c j) -> p c j", c=32),
                tmask_d.rearrange("c p j -> p c j"),
            )
            ident = const.tile([128, 128], F32)
            masks.make_identity(nc, ident[:])

            if loop > 1:
                with tc.For_i(0, loop, 1):
                    _emit_body(nc, tc, ctx, 0, qt, kt, vs, sm, tm, ident,
                               w_out, vec_out)
            else:
                for rep in range(repeat):
                    _emit_body(nc, tc, ctx, rep, qt, kt, vs, sm, tm, ident,
                               w_out, vec_out)

    nc.compile()
    return nc


def _emit_body(nc, tc, octx, rep, qt, kt, vs, sm, tm, ident, w_out, vec_out):
    with ExitStack() as ctx:
        wtp = ctx.enter_context(tc.tile_pool(name=f"wt{rep}", bufs=1))
        stg = ctx.enter_context(tc.tile_pool(name=f"stg{rep}", bufs=3))
        vtp = ctx.enter_context(tc.tile_pool(name=f"vt{rep}", bufs=2))
        vcp = ctx.enter_context(tc.tile_pool(name=f"vc{rep}", bufs=2))
        smallp = ctx.enter_context(tc.tile_pool(name=f"small{rep}", bufs=4))
        rp = ctx.enter_context(tc.tile_pool(name=f"rall{rep}", bufs=1))
        ps_sc = ctx.enter_context(
            tc.tile_pool(name=f"ps_sc{rep}", bufs=3, space="PSUM"))
        ps_vt = ctx.enter_context(
            tc.tile_pool(name=f"ps_vt{rep}", bufs=1, space="PSUM"))
        ps_tr = ctx.enter_context(
            tc.tile_pool(name=f"ps_tr{rep}", bufs=1, space="PSUM"))

        r_all = rp.tile([128, NLOCAL], F32)   # 1/rowsum per local block
        wt = {}  # ci -> bf16 tile [128, suffix]

        def emit_st_row(ci: int):
            """S^T pass for c-row ci: wT[ci] = exp(scores^T / 8) (bf16)."""
            kmin = ci // 2
            sfx = 2048 - 128 * kmin
            base = 128 * kmin
            t = wtp.tile([128, sfx], BF16, tag=f"wt{ci}")
            wt[ci] = t
            for (o, w1, w2) in _pair_chunks(sfx):
                fd = w1 + w2
                ps = ps_sc.tile([128, 1024], F32, tag="score")
                nc.tensor.matmul(
                    ps[:, 0:w1],
                    kt[0:64, 128 * ci:128 * ci + 128],
                    qt[0:64, base + o:base + o + w1],
                    start=True, stop=True,
                )
                if w2:
                    nc.tensor.matmul(
                        ps[:, 512:512 + w2],
                        kt[64:128, 128 * ci:128 * ci + 128],
                        qt[64:128, base + o + w1:base + o + w1 + w2],
                        start=True, stop=True,
                        tile_position=(64, 0),
                    )
                nc.scalar.activation(
                    t[:, o:o + fd], ps[:, 0:fd],
                    mybir.ActivationFunctionType.Exp, scale=SCALE,
                )
            # boundary sub-block: zero/tri/dense fixup (data-dependent mask)
            nc.vector.tensor_mul(
                t[:, 0:128], t[:, 0:128], tm[:, 128 * ci:128 * ci + 128])

        def emit_s_block(k: int):
            """S pass for local q-block k: normalized weights -> HBM."""
            W = _padded_width(k)
            stage = stg.tile([128, S], F32, tag="stage")
            partials = smallp.tile([128, 8], F32, tag="partials")
            prs = _pair_chunks(W)
            for cc, (o, w1, w2) in enumerate(prs):
                fd = w1 + w2
                ps = ps_sc.tile([128, 1024], F32, tag="score")
                nc.tensor.matmul(
                    ps[:, 0:w1],
                    qt[0:64, 128 * k:128 * k + 128],
                    kt[0:64, o:o + w1],
                    start=True, stop=True,
                )
                if w2:
                    nc.tensor.matmul(
                        ps[:, 512:512 + w2],
                        qt[64:128, 128 * k:128 * k + 128],
                        kt[64:128, o + w1:o + w1 + w2],
                        start=True, stop=True,
                        tile_position=(64, 0),
                    )
                nc.scalar.activation(
                    stage[:, o:o + fd], ps[:, 0:fd],
                    mybir.ActivationFunctionType.Exp, scale=SCALE,
                    accum_out=partials[:, cc:cc + 1],
                )
            # mask last 256 columns (triangle + padding) to exact 0
            nc.vector.tensor_mul(
                stage[:, W - 256:W], stage[:, W - 256:W],
                sm[:, 256 * k:256 * k + 256])
            # last pair's partial included masked garbage; recompute it
            o_l = prs[-1][0]
            fd_l = prs[-1][1] + prs[-1][2]
            ncc = len(prs)
            nc.vector.reduce_sum(
                out=partials[:, ncc - 1:ncc],
                in_=stage[:, o_l:o_l + fd_l],
                axis=mybir.AxisListType.X,
            )
            nc.vector.reduce_sum(
                out=partials[:, 7:8], in_=partials[:, 0:ncc],
                axis=mybir.AxisListType.X,
            )
            nc.vector.reciprocal(r_all[:, k:k + 1], partials[:, 7:8])
            nc.vector.tensor_scalar_mul(
                stage[:, 0:W], stage[:, 0:W], r_all[:, k:k + 1])
            nc.sync.dma_start(
                w_out[128 * k:128 * k + 128, 0:W], stage[:, 0:W])

        def emit_pv_window(w: int):
            """PV for local q window [512w, 512w+512): vec rows + DMA."""
            ci_hi = min(31, 8 * w + 7)
            pv = ps_vt.tile([65, 512], F32)
            n_ci = ci_hi + 1
            for ci in range(n_ci):
                o = 512 * w - 128 * (ci // 2)
                lhs = vs[:, 65 * ci:65 * ci + 65]
                if o >= 0:
                    nc.tensor.matmul(
                        pv[:, 0:512], lhs, wt[ci][:, o:o + 512],
                        start=(ci == 0), stop=(ci == n_ci - 1),
                    )
                else:
                    nc.tensor.matmul(
                        pv[:, -o:512], lhs, wt[ci][:, 0:512 + o],
                        start=False, stop=(ci == n_ci - 1),
                    )
            vt_sb = vtp.tile([65, 512], F32, tag="vtsb")
            nc.vector.tensor_copy(vt_sb[:], pv[:])
            for j in range(4):
                k = 4 * w + j
                tr = ps_tr.tile([128, 65], F32)
                nc.tensor.transpose(
                    tr[:], vt_sb[:, 128 * j:128 * j + 128],
                    ident[0:65, 0:65])
                rv = smallp.tile([128, 1], F32, tag="rv")
                nc.vector.reciprocal(rv[:], tr[:, 64:65])
                vec_sb = vcp.tile([128, 64], F32, tag="vecsb")
                nc.vector.tensor_scalar_mul(
                    vec_sb[:], tr[:, 0:64], rv[:])
                nc.sync.dma_start(
                    vec_out[128 * k:128 * k + 128, :], vec_sb[:])

        # ---- emission order: interleave for pipelining ----
        # S-blocks descend (largest first => big w DMAs start early);
        # S^T rows ascend (PV windows unlock progressively).
        for u in range(NLOCAL):
            emit_st_row(2 * u)
            emit_st_row(2 * u + 1)
            emit_s_block(NLOCAL - 1 - u)
            if u % 4 == 3:
                emit_pv_window(u // 4)


_NC = None


def _get_model():
    global _NC
    if _NC is None:
        _NC = build_model()
    return _NC


def _host_masks(parity: int):
    gl = _block_list(parity)
    smask = np.zeros((NLOCAL, 128, 256), dtype=np.float32)
    for k, g in enumerate(gl):
        W = _padded_width(k)
        cols = W - 256 + np.arange(256)[None, :]
        rows = 128 * g + np.arange(128)[:, None]
        smask[k] = (cols <= rows).astype(np.float32)
    smask = smask.astype(ml_dtypes.bfloat16)
    tmask = np.zeros((32, 128, 128), dtype=np.float32)
    for ci in range(32):
        kmin = ci // 2
        g = gl[kmin]
        rows_c = 128 * ci + np.arange(128)[:, None]
        cols_q = 128 * g + np.arange(128)[None, :]
        tmask[ci] = (rows_c <= cols_q)
    return smask, tmask.astype(ml_dtypes.bfloat16)


def make_in_maps(query, key, value):
    query = np.asarray(query, dtype=np.float32)
    key = np.asarray(key, dtype=np.float32)
    value = np.asarray(value, dtype=np.float32)
    mask_cache = {p: _host_masks(p) for p in (0, 1)}
    in_maps = []
    for c in range(NCORES):
        b, p = c // 2, c % 2
        gl = _block_list(p)
        rows = np.concatenate(
            [np.arange(128 * g, 128 * g + 128) for g in gl])
        qt1 = query[b][rows].T                                # [64, 2048]
        kt1 = key[b].T                                        # [64, 4096]
        qt = np.ascontiguousarray(np.concatenate([qt1, qt1], axis=0))
        kt = np.ascontiguousarray(np.concatenate([kt1, kt1], axis=0))
        v = np.concatenate(
            [value[b], np.ones((S, 1), np.float32)], axis=1
        ).astype(ml_dtypes.bfloat16)                          # [4096, 65]
        smask, tmask = mask_cache[p]
        in_maps.append(
            {"qt": qt, "kt": kt, "v": v, "smask": smask, "tmask": tmask})
    return in_maps


def kernel(query, key, value):
    nc = _get_model()
    in_maps = make_in_maps(query, key, value)
    res = run_bass_kernel_spmd(nc, in_maps, core_ids=list(range(NCORES)))

    attn_vec = np.empty((B, S, D), dtype=np.float32)
    attn_w = np.empty((B, S, S), dtype=np.float32)
    for c in range(NCORES):
        b, p = c // 2, c % 2
        gl = _block_list(p)
        wsh = res.results[c]["w"]
        vsh = res.results[c]["vec"]
        for k, g in enumerate(gl):
            attn_w[b, 128 * g:128 * g + 128, :] = wsh[128 * k:128 * k + 128]
            attn_vec[b, 128 * g:128 * g + 128, :] = vsh[128 * k:128 * k + 128]
    return attn_vec, attn_w


# revision 40
# speedup vs baseline: 6.0337x; 5.5271x over previous
"""Causal dot-product attention (B=4, S=4096, D=64) on 8 TRN2 NeuronCores.

Returns BOTH outputs of the reference: (attn_vec [B,S,D], attn_weights [B,S,S]).

Sharding: data-parallel over batch (4) x causal-balanced q-row interleave (2).
Core c handles batch b = c//2 and 16 of the 32 128-row q-blocks, chosen as
pairs (i, 31-i) so every core owns the same total causal area (load balance).

SPMD trick: the per-core *program* must be identical, but the causal widths of
a core's blocks differ between the two parities. The program computes padded
uniform widths W_k = 128*(2k+2) for the k-th (sorted) local block and applies a
per-core *data* mask (host-provided) to the boundary columns, which zeroes the
above-diagonal part exactly. Columns beyond W_k are never written: the PJRT
execution path donates zero-initialized output buffers, so the skipped region
is exactly 0.0 (= reference: exp((x-1e31)/8 - m) underflows to 0).

PE packing: the contraction dim is D=64, so two score matmuls are packed into
the 128x128 array as row-groups (tile_position (0,0) / (64,0)); Q^T and K^T are
host-duplicated onto partitions 64..127 to feed the second row-group. Each
packed pair fills a 2-bank PSUM tile -> one FD<=1024 exp on ScalarE.

Per-core pipeline:
  S^T pass: scores^T tiles [c128, q-chunk] = K^T-slice.T @ Q^T (PE, f32)
            -> exp(x/8) -> wT bf16 in SBUF (ACT), boundary sub-block masked
  PV pass:  vecT[64, 512] += V-slice.T @ wT-slice (PE, bf16), per 512-q window
            -> PE-transpose back to [q,64], scale by 1/rowsum (DVE)
  S pass:   scores tiles [q128, c-chunk] = Q^T-slice.T @ K^T (PE, f32)
            -> exp(x/8) f32 with accum_out row-sums (ACT) -> mask tail (DVE)
            -> scale by 1/rowsum (DVE) -> DMA out active columns only
"""

import os
import numpy as np
from contextlib import ExitStack

import ml_dtypes

from concourse import bacc, tile, mybir, masks
from concourse.bass_utils import run_bass_kernel_spmd

B, S, D = 4, 4096, 64
NCORES = 8
NLOCAL = 16          # 128-row q-blocks per core
SCALE = 0.125        # 1/sqrt(64)

BF16 = mybir.dt.bfloat16
F32 = mybir.dt.float32
F32R = mybir.dt.float32r  # fp32 storage, single-pass (TF32-like) PE compute


def _block_list(parity: int) -> list[int]:
    """Global 128-row block ids owned by a core of this parity, sorted in
    DESCENDING order (so causal widths shrink with local index and PV
    windows complete with small final contributions)."""
    ids = []
    for i in range(16):
        if i % 2 == parity:
            ids.append(i)
            ids.append(31 - i)
    return sorted(ids, reverse=True)


def _padded_width(k: int) -> int:
    # uniform (parity-independent) padded causal width of local block k
    # (descending-g ordering: widths shrink with k)
    return 128 * (32 - 2 * k)


def _pair_chunks(total: int) -> list[tuple[int, int, int]]:
    """Split [0, total) into (offset, w1, w2) packed-matmul pairs (w2 may be 0).
    Each wi <= 512; one pair -> one PSUM tile -> one exp of FD = w1 + w2."""
    out = []
    o = 0
    while o < total:
        w1 = min(512, total - o)
        w2 = min(512, total - o - w1)
        out.append((o, w1, w2))
        o += w1 + w2
    return out


def build_model(repeat: int = 1, loop: int = 1):
    nc = bacc.Bacc("TRN2", target_bir_lowering=False, debug=False,
                   num_devices=NCORES)

    # qt/kt carry two copies of the transposed tensor: partitions 0..63 and
    # 64..127 (feeds the second PE row-group of packed matmuls).
    qt_d = nc.dram_tensor("qt", [128, 2048], F32R, kind="ExternalInput").ap()
    kt_d = nc.dram_tensor("kt", [128, S], F32R, kind="ExternalInput").ap()
    # V with an appended ones column: the PV matmul then yields row-sums
    # (for vec normalization) in output partition 64 for free.
    v_d = nc.dram_tensor("v", [S, D + 1], BF16, kind="ExternalInput").ap()
    smask_d = nc.dram_tensor("smask", [NLOCAL, 128, 256], BF16,
                             kind="ExternalInput").ap()
    tmask_d = nc.dram_tensor("tmask", [32, 128, 128], BF16,
                             kind="ExternalInput").ap()
    w_out = nc.dram_tensor("w", [2048, S], F32, kind="ExternalOutput").ap()
    dbg_out = None
    if os.environ.get("DBG_WT") == "1":
        dbg_out = nc.dram_tensor("dbg", [128, 2048], BF16,
                                 kind="ExternalOutput").ap()
    vec_out = nc.dram_tensor("vec", [2048, D], F32, kind="ExternalOutput").ap()

    trace_sim = os.environ.get("TILE_TRACE_SIM") == "1"
    with tile.TileContext(nc, trace_sim=trace_sim) as tc:
        with ExitStack() as ctx:
            const = ctx.enter_context(tc.tile_pool(name="const", bufs=1))

            # ---- load inputs ----
            qt = const.tile([128, 2048], F32R)
            nc.sync.dma_start(qt[:], qt_d[:])
            kt = const.tile([128, S], F32R)
            nc.sync.dma_start(kt[:], kt_d[:])
            vs = const.tile([128, 32 * (D + 1)], BF16)
            nc.sync.dma_start(
                vs[:].rearrange("p (t d) -> p t d", t=32),
                v_d.rearrange("(t p) d -> p t d", p=128),
            )
            sm = const.tile([128, NLOCAL * 256], BF16)
            nc.sync.dma_start(
                sm[:].rearrange("p (k j) -> p k j", k=NLOCAL),
                smask_d.rearrange("k p j -> p k j"),
            )
            tm = const.tile([128, 32 * 128], BF16)
            nc.sync.dma_start(
                tm[:].rearrange("p (c j) -> p c j", c=32),
                tmask_d.rearrange("c p j -> p c j"),
            )
            ident = const.tile([128, 128], F32)
            masks.make_identity(nc, ident[:])

            if loop > 1:
                with tc.For_i(0, loop, 1):
                    _emit_body(nc, tc, ctx, 0, qt, kt, vs, sm, tm, ident,
                               w_out, vec_out, None)
            else:
                for rep in range(repeat):
                    _emit_body(nc, tc, ctx, rep, qt, kt, vs, sm, tm, ident,
                               w_out, vec_out, dbg_out)

    nc.compile()
    return nc


def _emit_body(nc, tc, octx, rep, qt, kt, vs, sm, tm, ident, w_out, vec_out, dbg_out=None):
    abl = os.environ.get("ABL", "")
    with ExitStack() as ctx:
        wtp = ctx.enter_context(tc.tile_pool(name=f"wt{rep}", bufs=1))
        stg = ctx.enter_context(tc.tile_pool(name=f"stg{rep}", bufs=4))
        vtp = ctx.enter_context(tc.tile_pool(name=f"vt{rep}", bufs=2))
        vcp = ctx.enter_context(tc.tile_pool(name=f"vc{rep}", bufs=2))
        smallp = ctx.enter_context(tc.tile_pool(name=f"small{rep}", bufs=4))
        rp = ctx.enter_context(tc.tile_pool(name=f"rall{rep}", bufs=1))
        ps_sc = ctx.enter_context(
            tc.tile_pool(name=f"ps_sc{rep}", bufs=3, space="PSUM"))
        ps_vt = ctx.enter_context(
            tc.tile_pool(name=f"ps_vt{rep}", bufs=2, space="PSUM"))

        r_all = rp.tile([128, NLOCAL], F32)   # 1/rowsum per local block
        vec_acc = rp.tile([128, NLOCAL * 64], F32)  # all vec blocks, 1 DMA
        wt = {}  # ci -> bf16 tile [128, suffix]

        def emit_st_row(ci: int):
            """S^T pass for c-row ci: wT[ci] = exp(scores^T / 8) (bf16).
            Active local q is the PREFIX [0, L) with L = 2048 - 128*(ci//2);
            the last 128 columns are the data-masked boundary block."""
            L = 2048 - 128 * (ci // 2)
            base = 0
            t = wtp.tile([128, L], BF16, tag=f"wt{ci}")
            wt[ci] = t
            for (o, w1, w2) in _pair_chunks(L):
                fd = w1 + w2
                ps = ps_sc.tile([128, 1024], F32, tag="score")
                nc.tensor.matmul(
                    ps[:, 0:w1],
                    kt[0:64, 128 * ci:128 * ci + 128],
                    qt[0:64, base + o:base + o + w1],
                    start=True, stop=True,
                )
                if w2:
                    nc.tensor.matmul(
                        ps[:, 512:512 + w2],
                        kt[64:128, 128 * ci:128 * ci + 128],
                        qt[64:128, base + o + w1:base + o + w1 + w2],
                        start=True, stop=True,
                        tile_position=(64, 0),
                    )
                if abl == "dupmm":
                    nc.tensor.matmul(
                        ps[:, 0:w1],
                        kt[0:64, 128 * ci:128 * ci + 128],
                        qt[0:64, base + o:base + o + w1],
                        start=True, stop=True,
                    )
                    if w2:
                        nc.tensor.matmul(
                            ps[:, 512:512 + w2],
                            kt[64:128, 128 * ci:128 * ci + 128],
                            qt[64:128, base + o + w1:base + o + w1 + w2],
                            start=True, stop=True,
                            tile_position=(64, 0),
                        )
                nc.scalar.activation(
                    t[:, o:o + fd], ps[:, 0:fd],
                    mybir.ActivationFunctionType.Exp, scale=SCALE,
                )
                if abl == "dupexp":
                    nc.scalar.activation(
                        t[:, o:o + fd], ps[:, 0:fd],
                        mybir.ActivationFunctionType.Exp, scale=SCALE,
                    )
            # boundary sub-block: zero/tri/dense fixup (data-dependent
            # mask) on the last 128 columns of the active prefix
            nc.vector.tensor_mul(
                t[:, L - 128:L], t[:, L - 128:L],
                tm[:, 128 * ci:128 * ci + 128])

        def emit_s_block(k: int):
            """S pass for local q-block k: normalized weights -> HBM."""
            W = _padded_width(k)
            stage = stg.tile([128, S], F32, tag="stage")
            partials = smallp.tile([128, 8], F32, tag="partials")
            prs = _pair_chunks(W)
            for cc, (o, w1, w2) in enumerate(prs):
                fd = w1 + w2
                ps = ps_sc.tile([128, 1024], F32, tag="score")
                nc.tensor.matmul(
                    ps[:, 0:w1],
                    qt[0:64, 128 * k:128 * k + 128],
                    kt[0:64, o:o + w1],
                    start=True, stop=True,
                )
                if w2:
                    nc.tensor.matmul(
                        ps[:, 512:512 + w2],
                        qt[64:128, 128 * k:128 * k + 128],
                        kt[64:128, o + w1:o + w1 + w2],
                        start=True, stop=True,
                        tile_position=(64, 0),
                    )
                if abl == "dupmm":
                    nc.tensor.matmul(
                        ps[:, 0:w1],
                        qt[0:64, 128 * k:128 * k + 128],
                        kt[0:64, o:o + w1],
                        start=True, stop=True,
                    )
                    if w2:
                        nc.tensor.matmul(
                            ps[:, 512:512 + w2],
                            qt[64:128, 128 * k:128 * k + 128],
                            kt[64:128, o + w1:o + w1 + w2],
                            start=True, stop=True,
                            tile_position=(64, 0),
                        )
                if o + fd == W:
                    # additive causal mask on the last 256 cols (psum local
                    # coords): masked scores -> -6e4 -> exp underflows to 0,
                    # so accum_out row-sums need no correction.
                    lo = W - 256 - o
                    nc.vector.tensor_add(
                        ps[:, lo:lo + 256], ps[:, lo:lo + 256],
                        sm[:, 256 * k:256 * k + 256])
                nc.scalar.activation(
                    stage[:, o:o + fd], ps[:, 0:fd],
                    mybir.ActivationFunctionType.Exp, scale=SCALE,
                    accum_out=partials[:, cc:cc + 1],
                )
                if abl == "dupexp":
                    nc.scalar.activation(
                        stage[:, o:o + fd], ps[:, 0:fd],
                        mybir.ActivationFunctionType.Exp, scale=SCALE,
                        accum_out=partials[:, cc:cc + 1],
                    )
            ncc = len(prs)
            nc.vector.reduce_sum(
                out=partials[:, 7:8], in_=partials[:, 0:ncc],
                axis=mybir.AxisListType.X,
            )
            nc.vector.reciprocal(r_all[:, k:k + 1], partials[:, 7:8])
            nc.vector.tensor_scalar_mul(
                stage[:, 0:W], stage[:, 0:W], r_all[:, k:k + 1])
            nc.sync.dma_start(
                w_out[128 * k:128 * k + 128, 0:W], stage[:, 0:W])
            if abl == "dupdma":
                nc.sync.dma_start(
                    w_out[128 * k:128 * k + 128, 0:W], stage[:, 0:W])

        pv_state = {}

        def emit_pv_chunk(w: int, ci_lo: int, ci_hi: int, last: bool):
            """Accumulate rows [ci_lo, ci_hi] of PV window w."""
            if w not in pv_state:
                pv_state[w] = ps_vt.tile([65, 512], F32, tag="vttr", name=f"pv{w}")
            pv = pv_state[w]
            for ci in range(ci_lo, ci_hi + 1):
                L = 2048 - 128 * (ci // 2)
                ww = min(512, L - 512 * w)
                lhs = vs[:, 65 * ci:65 * ci + 65]
                nc.tensor.matmul(
                    pv[:, 0:ww], lhs, wt[ci][:, 512 * w:512 * w + ww],
                    start=(ci == 0), stop=(last and ci == ci_hi),
                )

        def emit_pv_window(w: int, final: bool = False):
            """Finalize PV window w: transpose back to [q, d], scale; the
            last-finalized window DMAs the whole batched vec tile."""
            pv = pv_state.pop(w)
            vt_sb = vtp.tile([65, 512], F32, tag="vtsb")
            nc.vector.tensor_copy(vt_sb[:], pv[:])
            for j in range(4):
                k = 4 * w + j
                tr = ps_vt.tile([128, 65], F32, tag="vttr")
                nc.tensor.transpose(
                    tr[:], vt_sb[:, 128 * j:128 * j + 128],
                    ident[0:65, 0:65])
                rv = smallp.tile([128, 1], F32, tag="rv")
                nc.vector.reciprocal(rv[:], tr[:, 64:65])
                nc.vector.tensor_scalar_mul(
                    vec_acc[:, 64 * k:64 * k + 64], tr[:, 0:64], rv[:])
            if final:
                nc.sync.dma_start(
                    vec_out.rearrange("(k p) d -> p k d", p=128),
                    vec_acc[:].rearrange("p (k d) -> p k d", k=NLOCAL))

        # ---- emission order: interleave for pipelining ----
        # S-blocks descend (largest first => big w DMAs start early);
        # S^T rows ascend. PV windows start accumulating as soon as their
        # first rows exist (2 windows in flight), dribbling <=6 rows per
        # step to avoid PE bursts that would starve ScalarE; each window is
        # finalized at the earliest step its last row is available.
        pv_plan = [[] for _ in range(NLOCAL)]  # u -> [(w, lo, hi, last)]
        stages = [(3, 3, 3, 99), (2, 7, 7, 99), (1, 11, 11, 99), (0, 15, 15, 99)]
        for (w, u0, u1, per) in stages:
            lo = 0
            need = 32 - 8 * w
            for u in range(u0, u1 + 1):
                hi = min(need, lo + per)
                if hi > lo:
                    pv_plan[u].append((w, lo, hi - 1, hi == need))
                    lo = hi
        for u in range(NLOCAL):
            emit_st_row(2 * u)
            emit_st_row(2 * u + 1)
            for (w, lo, hi, last) in pv_plan[u]:
                emit_pv_chunk(w, lo, hi, last)
                if last:
                    emit_pv_window(w, final=(w == 0))
            emit_s_block(u)
        if dbg_out is not None:
            nc.sync.dma_start(dbg_out[:], wt[0][:])


_NC = None


def _get_model():
    global _NC
    if _NC is None:
        _NC = build_model()
    return _NC


def _host_masks(parity: int):
    gl = _block_list(parity)
    smask = np.zeros((NLOCAL, 128, 256), dtype=np.float32)
    for k, g in enumerate(gl):
        W = _padded_width(k)
        cols = W - 256 + np.arange(256)[None, :]
        rows = 128 * g + np.arange(128)[:, None]
        smask[k] = np.where(cols <= rows, 0.0, -60000.0)
    smask = smask.astype(ml_dtypes.bfloat16)
    tmask = np.zeros((32, 128, 128), dtype=np.float32)
    for ci in range(32):
        k_end = 15 - ci // 2
        g = gl[k_end]
        rows_c = 128 * ci + np.arange(128)[:, None]
        cols_q = 128 * g + np.arange(128)[None, :]
        tmask[ci] = (rows_c <= cols_q)
    return smask, tmask.astype(ml_dtypes.bfloat16)


def make_in_maps(query, key, value):
    query = np.asarray(query, dtype=np.float32)
    key = np.asarray(key, dtype=np.float32)
    value = np.asarray(value, dtype=np.float32)
    mask_cache = {p: _host_masks(p) for p in (0, 1)}
    in_maps = []
    for c in range(NCORES):
        b, p = c // 2, c % 2
        gl = _block_list(p)
        rows = np.concatenate(
            [np.arange(128 * g, 128 * g + 128) for g in gl])
        qt1 = query[b][rows].T                                # [64, 2048]
        kt1 = key[b].T                                        # [64, 4096]
        qt = np.ascontiguousarray(np.concatenate([qt1, qt1], axis=0))
        kt = np.ascontiguousarray(np.concatenate([kt1, kt1], axis=0))
        v = np.concatenate(
            [value[b], np.ones((S, 1), np.float32)], axis=1
        ).astype(ml_dtypes.bfloat16)                          # [4096, 65]
        smask, tmask = mask_cache[p]
        in_maps.append(
            {"qt": qt, "kt": kt, "v": v, "smask": smask, "tmask": tmask})
    return in_maps


def kernel(query, key, value):
    nc = _get_model()
    in_maps = make_in_maps(query, key, value)
    res = run_bass_kernel_spmd(nc, in_maps, core_ids=list(range(NCORES)))

    attn_vec = np.empty((B, S, D), dtype=np.float32)
    attn_w = np.empty((B, S, S), dtype=np.float32)
    for c in range(NCORES):
        b, p = c // 2, c % 2
        gl = _block_list(p)
        wsh = res.results[c]["w"]
        vsh = res.results[c]["vec"]
        for k, g in enumerate(gl):
            attn_w[b, 128 * g:128 * g + 128, :] = wsh[128 * k:128 * k + 128]
            attn_vec[b, 128 * g:128 * g + 128, :] = vsh[128 * k:128 * k + 128]
    return attn_vec, attn_w


# revision 41
# speedup vs baseline: 6.0469x; 1.0022x over previous
"""Causal dot-product attention (B=4, S=4096, D=64) on 8 TRN2 NeuronCores.

Returns BOTH outputs of the reference: (attn_vec [B,S,D], attn_weights [B,S,S]).

Sharding: data-parallel over batch (4) x causal-balanced q-row interleave (2).
Core c handles batch b = c//2 and 16 of the 32 128-row q-blocks, chosen as
pairs (i, 31-i) so every core owns the same total causal area (load balance).

SPMD trick: the per-core *program* must be identical, but the causal widths of
a core's blocks differ between the two parities. The program computes padded
uniform widths W_k = 128*(2k+2) for the k-th (sorted) local block and applies a
per-core *data* mask (host-provided) to the boundary columns, which zeroes the
above-diagonal part exactly. Columns beyond W_k are never written: the PJRT
execution path donates zero-initialized output buffers, so the skipped region
is exactly 0.0 (= reference: exp((x-1e31)/8 - m) underflows to 0).

PE packing: the contraction dim is D=64, so two score matmuls are packed into
the 128x128 array as row-groups (tile_position (0,0) / (64,0)); Q^T and K^T are
host-duplicated onto partitions 64..127 to feed the second row-group. Each
packed pair fills a 2-bank PSUM tile -> one FD<=1024 exp on ScalarE.

Per-core pipeline:
  S^T pass: scores^T tiles [c128, q-chunk] = K^T-slice.T @ Q^T (PE, f32)
            -> exp(x/8) -> wT bf16 in SBUF (ACT), boundary sub-block masked
  PV pass:  vecT[64, 512] += V-slice.T @ wT-slice (PE, bf16), per 512-q window
            -> PE-transpose back to [q,64], scale by 1/rowsum (DVE)
  S pass:   scores tiles [q128, c-chunk] = Q^T-slice.T @ K^T (PE, f32)
            -> exp(x/8) f32 with accum_out row-sums (ACT) -> mask tail (DVE)
            -> scale by 1/rowsum (DVE) -> DMA out active columns only
"""

import os
import numpy as np
from contextlib import ExitStack

import ml_dtypes

from concourse import bacc, tile, mybir, masks
from concourse.bass_utils import run_bass_kernel_spmd

B, S, D = 4, 4096, 64
NCORES = 8
NLOCAL = 16          # 128-row q-blocks per core
SCALE = 0.125        # 1/sqrt(64)

BF16 = mybir.dt.bfloat16
F32 = mybir.dt.float32
F32R = mybir.dt.float32r  # fp32 storage, single-pass (TF32-like) PE compute


def _block_list(parity: int) -> list[int]:
    """Global 128-row block ids owned by a core of this parity, sorted in
    DESCENDING order (so causal widths shrink with local index and PV
    windows complete with small final contributions)."""
    ids = []
    for i in range(16):
        if i % 2 == parity:
            ids.append(i)
            ids.append(31 - i)
    return sorted(ids, reverse=True)


def _padded_width(k: int) -> int:
    # uniform (parity-independent) padded causal width of local block k
    # (descending-g ordering: widths shrink with k)
    return 128 * (32 - 2 * k)


def _pair_chunks(total: int) -> list[tuple[int, int, int]]:
    """Split [0, total) into (offset, w1, w2) packed-matmul pairs (w2 may be 0).
    Each wi <= 512; one pair -> one PSUM tile -> one exp of FD = w1 + w2."""
    out = []
    o = 0
    while o < total:
        w1 = min(512, total - o)
        w2 = min(512, total - o - w1)
        out.append((o, w1, w2))
        o += w1 + w2
    return out


def build_model(repeat: int = 1, loop: int = 1):
    nc = bacc.Bacc("TRN2", target_bir_lowering=False, debug=False,
                   num_devices=NCORES)

    # qt/kt carry two copies of the transposed tensor: partitions 0..63 and
    # 64..127 (feeds the second PE row-group of packed matmuls).
    qt_d = nc.dram_tensor("qt", [128, 2048], F32R, kind="ExternalInput").ap()
    kt_d = nc.dram_tensor("kt", [128, S], F32R, kind="ExternalInput").ap()
    # V with an appended ones column: the PV matmul then yields row-sums
    # (for vec normalization) in output partition 64 for free.
    v_d = nc.dram_tensor("v", [S, D + 1], BF16, kind="ExternalInput").ap()
    smask_d = nc.dram_tensor("smask", [NLOCAL, 128, 256], BF16,
                             kind="ExternalInput").ap()
    tmask_d = nc.dram_tensor("tmask", [32, 128, 128], BF16,
                             kind="ExternalInput").ap()
    w_out = nc.dram_tensor("w", [2048, S], F32, kind="ExternalOutput").ap()
    dbg_out = None
    if os.environ.get("DBG_WT") == "1":
        dbg_out = nc.dram_tensor("dbg", [128, 2048], BF16,
                                 kind="ExternalOutput").ap()
    vec_out = nc.dram_tensor("vec", [2048, D], F32, kind="ExternalOutput").ap()

    trace_sim = os.environ.get("TILE_TRACE_SIM") == "1"
    with tile.TileContext(nc, trace_sim=trace_sim) as tc:
        with ExitStack() as ctx:
            const = ctx.enter_context(tc.tile_pool(name="const", bufs=1))

            # ---- load inputs ----
            qt = const.tile([128, 2048], F32R)
            nc.sync.dma_start(qt[:], qt_d[:])
            kt = const.tile([128, S], F32R)
            nc.sync.dma_start(kt[:], kt_d[:])
            vs = const.tile([128, 32 * (D + 1)], BF16)
            nc.sync.dma_start(
                vs[:].rearrange("p (t d) -> p t d", t=32),
                v_d.rearrange("(t p) d -> p t d", p=128),
            )
            sm = const.tile([128, NLOCAL * 256], BF16)
            nc.sync.dma_start(
                sm[:].rearrange("p (k j) -> p k j", k=NLOCAL),
                smask_d.rearrange("k p j -> p k j"),
            )
            tm = const.tile([128, 32 * 128], BF16)
            nc.sync.dma_start(
                tm[:].rearrange("p (c j) -> p c j", c=32),
                tmask_d.rearrange("c p j -> p c j"),
            )
            ident = const.tile([128, 128], F32)
            masks.make_identity(nc, ident[:])

            if loop > 1:
                with tc.For_i(0, loop, 1):
                    _emit_body(nc, tc, ctx, 0, qt, kt, vs, sm, tm, ident,
                               w_out, vec_out, None)
            else:
                for rep in range(repeat):
                    _emit_body(nc, tc, ctx, rep, qt, kt, vs, sm, tm, ident,
                               w_out, vec_out, dbg_out)

    nc.compile()
    return nc


def _emit_body(nc, tc, octx, rep, qt, kt, vs, sm, tm, ident, w_out, vec_out, dbg_out=None):
    abl = os.environ.get("ABL", "")
    with ExitStack() as ctx:
        wtp = ctx.enter_context(tc.tile_pool(name=f"wt{rep}", bufs=1))
        stg_a = ctx.enter_context(tc.tile_pool(name=f"stgA{rep}", bufs=2))
        stg_b = ctx.enter_context(tc.tile_pool(name=f"stgB{rep}", bufs=3))
        vtp = ctx.enter_context(tc.tile_pool(name=f"vt{rep}", bufs=2))
        vcp = ctx.enter_context(tc.tile_pool(name=f"vc{rep}", bufs=2))
        smallp = ctx.enter_context(tc.tile_pool(name=f"small{rep}", bufs=4))
        rp = ctx.enter_context(tc.tile_pool(name=f"rall{rep}", bufs=1))
        ps_sc = ctx.enter_context(
            tc.tile_pool(name=f"ps_sc{rep}", bufs=3, space="PSUM"))
        ps_vt = ctx.enter_context(
            tc.tile_pool(name=f"ps_vt{rep}", bufs=2, space="PSUM"))

        r_all = rp.tile([128, NLOCAL], F32)   # 1/rowsum per local block
        vec_acc = rp.tile([128, NLOCAL * 64], F32)  # all vec blocks, 1 DMA
        wt = {}  # ci -> bf16 tile [128, suffix]

        def emit_st_row(ci: int):
            """S^T pass for c-row ci: wT[ci] = exp(scores^T / 8) (bf16).
            Active local q is the PREFIX [0, L) with L = 2048 - 128*(ci//2);
            the last 128 columns are the data-masked boundary block."""
            L = 2048 - 128 * (ci // 2)
            base = 0
            t = wtp.tile([128, L], BF16, tag=f"wt{ci}")
            wt[ci] = t
            for (o, w1, w2) in _pair_chunks(L):
                fd = w1 + w2
                ps = ps_sc.tile([128, 1024], F32, tag="score")
                nc.tensor.matmul(
                    ps[:, 0:w1],
                    kt[0:64, 128 * ci:128 * ci + 128],
                    qt[0:64, base + o:base + o + w1],
                    start=True, stop=True,
                )
                if w2:
                    nc.tensor.matmul(
                        ps[:, 512:512 + w2],
                        kt[64:128, 128 * ci:128 * ci + 128],
                        qt[64:128, base + o + w1:base + o + w1 + w2],
                        start=True, stop=True,
                        tile_position=(64, 0),
                    )
                if abl == "dupmm":
                    nc.tensor.matmul(
                        ps[:, 0:w1],
                        kt[0:64, 128 * ci:128 * ci + 128],
                        qt[0:64, base + o:base + o + w1],
                        start=True, stop=True,
                    )
                    if w2:
                        nc.tensor.matmul(
                            ps[:, 512:512 + w2],
                            kt[64:128, 128 * ci:128 * ci + 128],
                            qt[64:128, base + o + w1:base + o + w1 + w2],
                            start=True, stop=True,
                            tile_position=(64, 0),
                        )
                nc.scalar.activation(
                    t[:, o:o + fd], ps[:, 0:fd],
                    mybir.ActivationFunctionType.Exp, scale=SCALE,
                )
                if abl == "dupexp":
                    nc.scalar.activation(
                        t[:, o:o + fd], ps[:, 0:fd],
                        mybir.ActivationFunctionType.Exp, scale=SCALE,
                    )
            # boundary sub-block: zero/tri/dense fixup (data-dependent
            # mask) on the last 128 columns of the active prefix
            nc.vector.tensor_mul(
                t[:, L - 128:L], t[:, L - 128:L],
                tm[:, 128 * ci:128 * ci + 128])

        def emit_s_block(k: int):
            """S pass for local q-block k: normalized weights -> HBM."""
            W = _padded_width(k)
            if W > 2048:
                stage = stg_a.tile([128, S], F32, tag="stageA")
            else:
                stage = stg_b.tile([128, 2048], F32, tag="stageB")
            partials = smallp.tile([128, 8], F32, tag="partials")
            prs = _pair_chunks(W)
            for cc, (o, w1, w2) in enumerate(prs):
                fd = w1 + w2
                ps = ps_sc.tile([128, 1024], F32, tag="score")
                nc.tensor.matmul(
                    ps[:, 0:w1],
                    qt[0:64, 128 * k:128 * k + 128],
                    kt[0:64, o:o + w1],
                    start=True, stop=True,
                )
                if w2:
                    nc.tensor.matmul(
                        ps[:, 512:512 + w2],
                        qt[64:128, 128 * k:128 * k + 128],
                        kt[64:128, o + w1:o + w1 + w2],
                        start=True, stop=True,
                        tile_position=(64, 0),
                    )
                if abl == "dupmm":
                    nc.tensor.matmul(
                        ps[:, 0:w1],
                        qt[0:64, 128 * k:128 * k + 128],
                        kt[0:64, o:o + w1],
                        start=True, stop=True,
                    )
                    if w2:
                        nc.tensor.matmul(
                            ps[:, 512:512 + w2],
                            qt[64:128, 128 * k:128 * k + 128],
                            kt[64:128, o + w1:o + w1 + w2],
                            start=True, stop=True,
                            tile_position=(64, 0),
                        )
                if o + fd == W:
                    # additive causal mask on the last 256 cols (psum local
                    # coords): masked scores -> -6e4 -> exp underflows to 0,
                    # so accum_out row-sums need no correction.
                    lo = W - 256 - o
                    nc.vector.tensor_add(
                        ps[:, lo:lo + 256], ps[:, lo:lo + 256],
                        sm[:, 256 * k:256 * k + 256])
                nc.scalar.activation(
                    stage[:, o:o + fd], ps[:, 0:fd],
                    mybir.ActivationFunctionType.Exp, scale=SCALE,
                    accum_out=partials[:, cc:cc + 1],
                )
                if abl == "dupexp":
                    nc.scalar.activation(
                        stage[:, o:o + fd], ps[:, 0:fd],
                        mybir.ActivationFunctionType.Exp, scale=SCALE,
                        accum_out=partials[:, cc:cc + 1],
                    )
            ncc = len(prs)
            nc.vector.reduce_sum(
                out=partials[:, 7:8], in_=partials[:, 0:ncc],
                axis=mybir.AxisListType.X,
            )
            nc.vector.reciprocal(r_all[:, k:k + 1], partials[:, 7:8])
            h = (W // 2) // 512 * 512
            if h == 0:
                h = W
            nc.vector.tensor_scalar_mul(
                stage[:, 0:h], stage[:, 0:h], r_all[:, k:k + 1])
            nc.sync.dma_start(
                w_out[128 * k:128 * k + 128, 0:h], stage[:, 0:h])
            if h < W:
                nc.vector.tensor_scalar_mul(
                    stage[:, h:W], stage[:, h:W], r_all[:, k:k + 1])
                nc.sync.dma_start(
                    w_out[128 * k:128 * k + 128, h:W], stage[:, h:W])
            if abl == "dupdma":
                nc.sync.dma_start(
                    w_out[128 * k:128 * k + 128, 0:W], stage[:, 0:W])

        pv_state = {}

        def emit_pv_chunk(w: int, ci_lo: int, ci_hi: int, last: bool):
            """Accumulate rows [ci_lo, ci_hi] of PV window w."""
            if w not in pv_state:
                pv_state[w] = ps_vt.tile([65, 512], F32, tag="vttr", name=f"pv{w}")
            pv = pv_state[w]
            for ci in range(ci_lo, ci_hi + 1):
                L = 2048 - 128 * (ci // 2)
                ww = min(512, L - 512 * w)
                lhs = vs[:, 65 * ci:65 * ci + 65]
                nc.tensor.matmul(
                    pv[:, 0:ww], lhs, wt[ci][:, 512 * w:512 * w + ww],
                    start=(ci == 0), stop=(last and ci == ci_hi),
                )

        def emit_pv_window(w: int, final: bool = False):
            """Finalize PV window w: transpose back to [q, d], scale; the
            last-finalized window DMAs the whole batched vec tile."""
            pv = pv_state.pop(w)
            vt_sb = vtp.tile([65, 512], F32, tag="vtsb")
            nc.vector.tensor_copy(vt_sb[:], pv[:])
            for j in range(4):
                k = 4 * w + j
                tr = ps_vt.tile([128, 65], F32, tag="vttr")
                nc.tensor.transpose(
                    tr[:], vt_sb[:, 128 * j:128 * j + 128],
                    ident[0:65, 0:65])
                rv = smallp.tile([128, 1], F32, tag="rv")
                nc.vector.reciprocal(rv[:], tr[:, 64:65])
                nc.vector.tensor_scalar_mul(
                    vec_acc[:, 64 * k:64 * k + 64], tr[:, 0:64], rv[:])
            if final:
                nc.sync.dma_start(
                    vec_out.rearrange("(k p) d -> p k d", p=128),
                    vec_acc[:].rearrange("p (k d) -> p k d", k=NLOCAL))

        # ---- emission order: interleave for pipelining ----
        # S-blocks descend (largest first => big w DMAs start early);
        # S^T rows ascend. PV windows start accumulating as soon as their
        # first rows exist (2 windows in flight), dribbling <=6 rows per
        # step to avoid PE bursts that would starve ScalarE; each window is
        # finalized at the earliest step its last row is available.
        pv_plan = [[] for _ in range(NLOCAL)]  # u -> [(w, lo, hi, last)]
        stages = [(3, 3, 3, 99), (2, 7, 7, 99), (1, 11, 11, 99), (0, 15, 15, 99)]
        for (w, u0, u1, per) in stages:
            lo = 0
            need = 32 - 8 * w
            for u in range(u0, u1 + 1):
                hi = min(need, lo + per)
                if hi > lo:
                    pv_plan[u].append((w, lo, hi - 1, hi == need))
                    lo = hi
        for u in range(NLOCAL):
            emit_st_row(2 * u)
            emit_st_row(2 * u + 1)
            for (w, lo, hi, last) in pv_plan[u]:
                emit_pv_chunk(w, lo, hi, last)
                if last:
                    emit_pv_window(w, final=(w == 0))
            emit_s_block(u)
        if dbg_out is not None:
            nc.sync.dma_start(dbg_out[:], wt[0][:])


_NC = None


def _get_model():
    global _NC
    if _NC is None:
        _NC = build_model()
    return _NC


def _host_masks(parity: int):
    gl = _block_list(parity)
    smask = np.zeros((NLOCAL, 128, 256), dtype=np.float32)
    for k, g in enumerate(gl):
        W = _padded_width(k)
        cols = W - 256 + np.arange(256)[None, :]
        rows = 128 * g + np.arange(128)[:, None]
        smask[k] = np.where(cols <= rows, 0.0, -60000.0)
    smask = smask.astype(ml_dtypes.bfloat16)
    tmask = np.zeros((32, 128, 128), dtype=np.float32)
    for ci in range(32):
        k_end = 15 - ci // 2
        g = gl[k_end]
        rows_c = 128 * ci + np.arange(128)[:, None]
        cols_q = 128 * g + np.arange(128)[None, :]
        tmask[ci] = (rows_c <= cols_q)
    return smask, tmask.astype(ml_dtypes.bfloat16)


def make_in_maps(query, key, value):
    query = np.asarray(query, dtype=np.float32)
    key = np.asarray(key, dtype=np.float32)
    value = np.asarray(value, dtype=np.float32)
    mask_cache = {p: _host_masks(p) for p in (0, 1)}
    in_maps = []
    for c in range(NCORES):
        b, p = c // 2, c % 2
        gl = _block_list(p)
        rows = np.concatenate(
            [np.arange(128 * g, 128 * g + 128) for g in gl])
        qt1 = query[b][rows].T                                # [64, 2048]
        kt1 = key[b].T                                        # [64, 4096]
        qt = np.ascontiguousarray(np.concatenate([qt1, qt1], axis=0))
        kt = np.ascontiguousarray(np.concatenate([kt1, kt1], axis=0))
        v = np.concatenate(
            [value[b], np.ones((S, 1), np.float32)], axis=1
        ).astype(ml_dtypes.bfloat16)                          # [4096, 65]
        smask, tmask = mask_cache[p]
        in_maps.append(
            {"qt": qt, "kt": kt, "v": v, "smask": smask, "tmask": tmask})
    return in_maps


def kernel(query, key, value):
    nc = _get_model()
    in_maps = make_in_maps(query, key, value)
    res = run_bass_kernel_spmd(nc, in_maps, core_ids=list(range(NCORES)))

    attn_vec = np.empty((B, S, D), dtype=np.float32)
    attn_w = np.empty((B, S, S), dtype=np.float32)
    for c in range(NCORES):
        b, p = c // 2, c % 2
        gl = _block_list(p)
        wsh = res.results[c]["w"]
        vsh = res.results[c]["vec"]
        for k, g in enumerate(gl):
            attn_w[b, 128 * g:128 * g + 128, :] = wsh[128 * k:128 * k + 128]
            attn_vec[b, 128 * g:128 * g + 128, :] = vsh[128 * k:128 * k + 128]
    return attn_vec, attn_w


# revision 42
# speedup vs baseline: 8.0880x; 1.3375x over previous
"""Causal dot-product attention (B=4, S=4096, D=64) on 8 TRN2 NeuronCores.

Returns BOTH outputs of the reference: (attn_vec [B,S,D], attn_weights [B,S,S]).

Sharding: data-parallel over batch (4) x causal-balanced q-row interleave (2).
Core c handles batch b = c//2 and 16 of the 32 128-row q-blocks, chosen as
pairs (i, 31-i) so every core owns the same total causal area (load balance).

SPMD trick: the per-core *program* must be identical, but the causal widths of
a core's blocks differ between the two parities. The program computes padded
uniform widths W_k = 128*(2k+2) for the k-th (sorted) local block and applies a
per-core *data* mask (host-provided) to the boundary columns, which zeroes the
above-diagonal part exactly. Columns beyond W_k are never written: the PJRT
execution path donates zero-initialized output buffers, so the skipped region
is exactly 0.0 (= reference: exp((x-1e31)/8 - m) underflows to 0).

PE packing: the contraction dim is D=64, so two score matmuls are packed into
the 128x128 array as row-groups (tile_position (0,0) / (64,0)); Q^T and K^T are
host-duplicated onto partitions 64..127 to feed the second row-group. Each
packed pair fills a 2-bank PSUM tile -> one FD<=1024 exp on ScalarE.

Per-core pipeline:
  S^T pass: scores^T tiles [c128, q-chunk] = K^T-slice.T @ Q^T (PE, f32)
            -> exp(x/8) -> wT bf16 in SBUF (ACT), boundary sub-block masked
  PV pass:  vecT[64, 512] += V-slice.T @ wT-slice (PE, bf16), per 512-q window
            -> PE-transpose back to [q,64], scale by 1/rowsum (DVE)
  S pass:   scores tiles [q128, c-chunk] = Q^T-slice.T @ K^T (PE, f32)
            -> exp(x/8) f32 with accum_out row-sums (ACT) -> mask tail (DVE)
            -> scale by 1/rowsum (DVE) -> DMA out active columns only
"""

import os
import numpy as np
from contextlib import ExitStack

import ml_dtypes

from concourse import bacc, tile, mybir, masks
from concourse.bass_utils import run_bass_kernel_spmd

B, S, D = 4, 4096, 64
NCORES = 8
NLOCAL = 16          # 128-row q-blocks per core
SCALE = 0.125        # 1/sqrt(64)

BF16 = mybir.dt.bfloat16
F32 = mybir.dt.float32
F32R = mybir.dt.float32r  # fp32 storage, single-pass (TF32-like) PE compute


def _block_list(parity: int) -> list[int]:
    """Global 128-row block ids owned by a core of this parity, sorted in
    DESCENDING order (so causal widths shrink with local index and PV
    windows complete with small final contributions)."""
    ids = []
    for i in range(16):
        if i % 2 == parity:
            ids.append(i)
            ids.append(31 - i)
    return sorted(ids, reverse=True)


def _padded_width(k: int) -> int:
    # uniform (parity-independent) padded causal width of local block k
    # (descending-g ordering: widths shrink with k)
    return 128 * (32 - 2 * k)


def _pair_chunks(total: int) -> list[tuple[int, int, int]]:
    """Split [0, total) into (offset, w1, w2) packed-matmul pairs (w2 may be 0).
    Each wi <= 512; one pair -> one PSUM tile -> one exp of FD = w1 + w2."""
    out = []
    o = 0
    while o < total:
        w1 = min(512, total - o)
        w2 = min(512, total - o - w1)
        out.append((o, w1, w2))
        o += w1 + w2
    return out


def build_model(repeat: int = 1, loop: int = 1):
    nc = bacc.Bacc("TRN2", target_bir_lowering=False, debug=False,
                   num_devices=NCORES)

    # qt/kt carry two copies of the transposed tensor: partitions 0..63 and
    # 64..127 (feeds the second PE row-group of packed matmuls).
    qt_d = nc.dram_tensor("qt", [128, 2048], F32R, kind="ExternalInput").ap()
    kt_d = nc.dram_tensor("kt", [128, S], F32R, kind="ExternalInput").ap()
    # V with an appended ones column: the PV matmul then yields row-sums
    # (for vec normalization) in output partition 64 for free.
    v_d = nc.dram_tensor("v", [S, D + 1], BF16, kind="ExternalInput").ap()
    smask_d = nc.dram_tensor("smask", [NLOCAL, 128, 256], BF16,
                             kind="ExternalInput").ap()
    tmask_d = nc.dram_tensor("tmask", [32, 128, 128], BF16,
                             kind="ExternalInput").ap()
    w_out = nc.dram_tensor("w", [2048, S], F32, kind="ExternalOutput").ap()
    dbg_out = None
    if os.environ.get("DBG_WT") == "1":
        dbg_out = nc.dram_tensor("dbg", [128, 2048], BF16,
                                 kind="ExternalOutput").ap()
    vec_out = nc.dram_tensor("vec", [2048, D], F32, kind="ExternalOutput").ap()

    trace_sim = os.environ.get("TILE_TRACE_SIM") == "1"
    with tile.TileContext(nc, trace_sim=trace_sim) as tc:
        with ExitStack() as ctx:
            const = ctx.enter_context(tc.tile_pool(name="const", bufs=1))

            # ---- load inputs ----
            qt = const.tile([128, 2048], F32R)
            nc.sync.dma_start(qt[:], qt_d[:])
            kt = const.tile([128, S], F32R)
            nc.sync.dma_start(kt[:], kt_d[:])
            vs = const.tile([128, 32 * (D + 1)], BF16)
            nc.sync.dma_start(
                vs[:].rearrange("p (t d) -> p t d", t=32),
                v_d.rearrange("(t p) d -> p t d", p=128),
            )
            sm = const.tile([128, NLOCAL * 256], BF16)
            nc.sync.dma_start(
                sm[:].rearrange("p (k j) -> p k j", k=NLOCAL),
                smask_d.rearrange("k p j -> p k j"),
            )
            tm = const.tile([128, 32 * 128], BF16)
            nc.sync.dma_start(
                tm[:].rearrange("p (c j) -> p c j", c=32),
                tmask_d.rearrange("c p j -> p c j"),
            )
            ident = const.tile([128, 128], F32)
            masks.make_identity(nc, ident[:])

            if loop > 1:
                with tc.For_i(0, loop, 1):
                    _emit_body(nc, tc, ctx, 0, qt, kt, vs, sm, tm, ident,
                               w_out, vec_out, None)
            else:
                for rep in range(repeat):
                    _emit_body(nc, tc, ctx, rep, qt, kt, vs, sm, tm, ident,
                               w_out, vec_out, dbg_out)

    nc.compile()
    return nc


def _emit_body(nc, tc, octx, rep, qt, kt, vs, sm, tm, ident, w_out, vec_out, dbg_out=None):
    abl = os.environ.get("ABL", "")
    with ExitStack() as ctx:
        wtp = ctx.enter_context(tc.tile_pool(name=f"wt{rep}", bufs=1))
        stg_a = ctx.enter_context(tc.tile_pool(name=f"stgA{rep}", bufs=2))
        stg_b = ctx.enter_context(tc.tile_pool(name=f"stgB{rep}", bufs=3))
        vtp = ctx.enter_context(tc.tile_pool(name=f"vt{rep}", bufs=2))
        vcp = ctx.enter_context(tc.tile_pool(name=f"vc{rep}", bufs=2))
        smallp = ctx.enter_context(tc.tile_pool(name=f"small{rep}", bufs=4))
        rp = ctx.enter_context(tc.tile_pool(name=f"rall{rep}", bufs=1))
        ps_sc = ctx.enter_context(
            tc.tile_pool(name=f"ps_sc{rep}", bufs=3, space="PSUM"))
        ps_vt = ctx.enter_context(
            tc.tile_pool(name=f"ps_vt{rep}", bufs=2, space="PSUM"))

        r_all = rp.tile([128, NLOCAL], F32)   # 1/rowsum per local block
        vec_acc = rp.tile([128, NLOCAL * 64], F32)  # all vec blocks, 1 DMA
        wt = {}  # ci -> bf16 tile [128, suffix]

        def emit_st_row(ci: int):
            """S^T pass for c-row ci: wT[ci] = exp(scores^T / 8) (bf16).
            Active local q is the PREFIX [0, L) with L = 2048 - 128*(ci//2);
            the last 128 columns are the data-masked boundary block."""
            L = 2048 - 128 * (ci // 2)
            base = 0
            t = wtp.tile([128, L], BF16, tag=f"wt{ci}")
            wt[ci] = t
            for (o, w1, w2) in _pair_chunks(L):
                fd = w1 + w2
                ps = ps_sc.tile([128, 1024], F32, tag="score")
                nc.tensor.matmul(
                    ps[:, 0:w1],
                    kt[0:64, 128 * ci:128 * ci + 128],
                    qt[0:64, base + o:base + o + w1],
                    start=True, stop=True,
                )
                if w2:
                    nc.tensor.matmul(
                        ps[:, 512:512 + w2],
                        kt[64:128, 128 * ci:128 * ci + 128],
                        qt[64:128, base + o + w1:base + o + w1 + w2],
                        start=True, stop=True,
                        tile_position=(64, 0),
                    )
                if abl == "dupmm":
                    nc.tensor.matmul(
                        ps[:, 0:w1],
                        kt[0:64, 128 * ci:128 * ci + 128],
                        qt[0:64, base + o:base + o + w1],
                        start=True, stop=True,
                    )
                    if w2:
                        nc.tensor.matmul(
                            ps[:, 512:512 + w2],
                            kt[64:128, 128 * ci:128 * ci + 128],
                            qt[64:128, base + o + w1:base + o + w1 + w2],
                            start=True, stop=True,
                            tile_position=(64, 0),
                        )
                nc.scalar.activation(
                    t[:, o:o + fd], ps[:, 0:fd],
                    mybir.ActivationFunctionType.Exp, scale=SCALE,
                )
                if abl == "dupexp":
                    nc.scalar.activation(
                        t[:, o:o + fd], ps[:, 0:fd],
                        mybir.ActivationFunctionType.Exp, scale=SCALE,
                    )
            # boundary sub-block: zero/tri/dense fixup (data-dependent
            # mask) on the last 128 columns of the active prefix
            nc.vector.tensor_mul(
                t[:, L - 128:L], t[:, L - 128:L],
                tm[:, 128 * ci:128 * ci + 128])

        def emit_s_block(k: int):
            """S pass for local q-block k: normalized weights -> HBM."""
            W = _padded_width(k)
            if W > 2048:
                stage = stg_a.tile([128, S], F32, tag="stageA")
            else:
                stage = stg_b.tile([128, 2048], F32, tag="stageB")
            partials = smallp.tile([128, 8], F32, tag="partials")
            prs = _pair_chunks(W)
            for cc, (o, w1, w2) in enumerate(prs):
                fd = w1 + w2
                ps = ps_sc.tile([128, 1024], F32, tag="score")
                nc.tensor.matmul(
                    ps[:, 0:w1],
                    qt[0:64, 128 * k:128 * k + 128],
                    kt[0:64, o:o + w1],
                    start=True, stop=True,
                )
                if w2:
                    nc.tensor.matmul(
                        ps[:, 512:512 + w2],
                        qt[64:128, 128 * k:128 * k + 128],
                        kt[64:128, o + w1:o + w1 + w2],
                        start=True, stop=True,
                        tile_position=(64, 0),
                    )
                if abl == "dupmm":
                    nc.tensor.matmul(
                        ps[:, 0:w1],
                        qt[0:64, 128 * k:128 * k + 128],
                        kt[0:64, o:o + w1],
                        start=True, stop=True,
                    )
                    if w2:
                        nc.tensor.matmul(
                            ps[:, 512:512 + w2],
                            qt[64:128, 128 * k:128 * k + 128],
                            kt[64:128, o + w1:o + w1 + w2],
                            start=True, stop=True,
                            tile_position=(64, 0),
                        )
                if o + fd == W:
                    # additive causal mask on the last 256 cols (psum local
                    # coords): masked scores -> -6e4 -> exp underflows to 0,
                    # so accum_out row-sums need no correction.
                    lo = W - 256 - o
                    nc.vector.tensor_add(
                        ps[:, lo:lo + 256], ps[:, lo:lo + 256],
                        sm[:, 256 * k:256 * k + 256])
                nc.scalar.activation(
                    stage[:, o:o + fd], ps[:, 0:fd],
                    mybir.ActivationFunctionType.Exp, scale=SCALE,
                    accum_out=partials[:, cc:cc + 1],
                )
                if abl == "dupexp":
                    nc.scalar.activation(
                        stage[:, o:o + fd], ps[:, 0:fd],
                        mybir.ActivationFunctionType.Exp, scale=SCALE,
                        accum_out=partials[:, cc:cc + 1],
                    )
            ncc = len(prs)
            nc.vector.reduce_sum(
                out=partials[:, 7:8], in_=partials[:, 0:ncc],
                axis=mybir.AxisListType.X,
            )
            nc.vector.reciprocal(r_all[:, k:k + 1], partials[:, 7:8])
            # split scale+DMA only for large blocks (early DMA start);
            # small blocks go out whole (fewer fixed DMA costs)
            h = (W // 2) // 512 * 512 if W > 2048 else W
            nc.vector.tensor_scalar_mul(
                stage[:, 0:h], stage[:, 0:h], r_all[:, k:k + 1])
            nc.sync.dma_start(
                w_out[128 * k:128 * k + 128, 0:h], stage[:, 0:h])
            if h < W:
                nc.vector.tensor_scalar_mul(
                    stage[:, h:W], stage[:, h:W], r_all[:, k:k + 1])
                nc.sync.dma_start(
                    w_out[128 * k:128 * k + 128, h:W], stage[:, h:W])
            if abl == "dupdma":
                nc.sync.dma_start(
                    w_out[128 * k:128 * k + 128, 0:W], stage[:, 0:W])

        pv_state = {}

        def emit_pv_chunk(w: int, ci_lo: int, ci_hi: int, last: bool):
            """Accumulate rows [ci_lo, ci_hi] of PV window w."""
            if w not in pv_state:
                pv_state[w] = ps_vt.tile([65, 512], F32, tag="vttr", name=f"pv{w}")
            pv = pv_state[w]
            for ci in range(ci_lo, ci_hi + 1):
                L = 2048 - 128 * (ci // 2)
                ww = min(512, L - 512 * w)
                lhs = vs[:, 65 * ci:65 * ci + 65]
                nc.tensor.matmul(
                    pv[:, 0:ww], lhs, wt[ci][:, 512 * w:512 * w + ww],
                    start=(ci == 0), stop=(last and ci == ci_hi),
                )

        def emit_pv_window(w: int, final: bool = False):
            """Finalize PV window w: transpose back to [q, d], scale; the
            last-finalized window DMAs the whole batched vec tile."""
            pv = pv_state.pop(w)
            vt_sb = vtp.tile([65, 512], F32, tag="vtsb")
            nc.vector.tensor_copy(vt_sb[:], pv[:])
            for j in range(4):
                k = 4 * w + j
                tr = ps_vt.tile([128, 65], F32, tag="vttr")
                nc.tensor.transpose(
                    tr[:], vt_sb[:, 128 * j:128 * j + 128],
                    ident[0:65, 0:65])
                rv = smallp.tile([128, 1], F32, tag="rv")
                nc.vector.reciprocal(rv[:], tr[:, 64:65])
                nc.vector.tensor_scalar_mul(
                    vec_acc[:, 64 * k:64 * k + 64], tr[:, 0:64], rv[:])
            if final:
                nc.sync.dma_start(
                    vec_out.rearrange("(k p) d -> p k d", p=128),
                    vec_acc[:].rearrange("p (k d) -> p k d", k=NLOCAL))

        # ---- emission order: interleave for pipelining ----
        # S-blocks descend (largest first => big w DMAs start early);
        # S^T rows ascend. PV windows start accumulating as soon as their
        # first rows exist (2 windows in flight), dribbling <=6 rows per
        # step to avoid PE bursts that would starve ScalarE; each window is
        # finalized at the earliest step its last row is available.
        pv_plan = [[] for _ in range(NLOCAL)]  # u -> [(w, lo, hi, last)]
        stages = [(3, 3, 3, 99), (2, 7, 7, 99), (1, 11, 11, 99), (0, 15, 15, 99)]
        for (w, u0, u1, per) in stages:
            lo = 0
            need = 32 - 8 * w
            for u in range(u0, u1 + 1):
                hi = min(need, lo + per)
                if hi > lo:
                    pv_plan[u].append((w, lo, hi - 1, hi == need))
                    lo = hi
        for u in range(NLOCAL):
            emit_st_row(2 * u)
            emit_st_row(2 * u + 1)
            for (w, lo, hi, last) in pv_plan[u]:
                emit_pv_chunk(w, lo, hi, last)
                if last:
                    emit_pv_window(w, final=(w == 0))
            emit_s_block(u)
        if dbg_out is not None:
            nc.sync.dma_start(dbg_out[:], wt[0][:])


_NC = None


def _get_model():
    global _NC
    if _NC is None:
        _NC = build_model()
    return _NC


def _host_masks(parity: int):
    gl = _block_list(parity)
    smask = np.zeros((NLOCAL, 128, 256), dtype=np.float32)
    for k, g in enumerate(gl):
        W = _padded_width(k)
        cols = W - 256 + np.arange(256)[None, :]
        rows = 128 * g + np.arange(128)[:, None]
        smask[k] = np.where(cols <= rows, 0.0, -60000.0)
    smask = smask.astype(ml_dtypes.bfloat16)
    tmask = np.zeros((32, 128, 128), dtype=np.float32)
    for ci in range(32):
        k_end = 15 - ci // 2
        g = gl[k_end]
        rows_c = 128 * ci + np.arange(128)[:, None]
        cols_q = 128 * g + np.arange(128)[None, :]
        tmask[ci] = (rows_c <= cols_q)
    return smask, tmask.astype(ml_dtypes.bfloat16)


def make_in_maps(query, key, value):
    query = np.asarray(query, dtype=np.float32)
    key = np.asarray(key, dtype=np.float32)
    value = np.asarray(value, dtype=np.float32)
    mask_cache = {p: _host_masks(p) for p in (0, 1)}
    in_maps = []
    for c in range(NCORES):
        b, p = c // 2, c % 2
        gl = _block_list(p)
        rows = np.concatenate(
            [np.arange(128 * g, 128 * g + 128) for g in gl])
        qt1 = query[b][rows].T                                # [64, 2048]
        kt1 = key[b].T                                        # [64, 4096]
        qt = np.ascontiguousarray(np.concatenate([qt1, qt1], axis=0))
        kt = np.ascontiguousarray(np.concatenate([kt1, kt1], axis=0))
        v = np.concatenate(
            [value[b], np.ones((S, 1), np.float32)], axis=1
        ).astype(ml_dtypes.bfloat16)                          # [4096, 65]
        smask, tmask = mask_cache[p]
        in_maps.append(
            {"qt": qt, "kt": kt, "v": v, "smask": smask, "tmask": tmask})
    return in_maps


def kernel(query, key, value):
    nc = _get_model()
    in_maps = make_in_maps(query, key, value)
    res = run_bass_kernel_spmd(nc, in_maps, core_ids=list(range(NCORES)))

    attn_vec = np.empty((B, S, D), dtype=np.float32)
    attn_w = np.empty((B, S, S), dtype=np.float32)
    for c in range(NCORES):
        b, p = c // 2, c % 2
        gl = _block_list(p)
        wsh = res.results[c]["w"]
        vsh = res.results[c]["vec"]
        for k, g in enumerate(gl):
            attn_w[b, 128 * g:128 * g + 128, :] = wsh[128 * k:128 * k + 128]
            attn_vec[b, 128 * g:128 * g + 128, :] = vsh[128 * k:128 * k + 128]
    return attn_vec, attn_w
